# revision 1
# baseline (speedup 1.0000x reference)
"""Trainium2 Bass kernel for a single attention head (B=4, S=2048, D=4096, DH=128).

Sharding: 8 cores = (batch b, q-half h). Each core:
  - computes K^T, V^T for the full sequence of its batch (duplicated across the
    2 cores of a batch), Q^T for its own 1024-row half,
  - logits (bf16-rounded, matching the reference) + mask -> softmax (fp32
    stats) -> bf16 weights -> out = W @ V, scaled by 1/rowsum.

Key layout trick: K/V sequence columns are PERMUTED on the host so each core's
own q-half comes first; the Q projection then always reads columns [0,1024) of
xT, keeping the SPMD graph identical across cores. Softmax/PV are invariant to
a consistent key permutation.

Phase structure (PSUM is 8 banks of [128,512]f32):
  1a: per d-chunk i: one [128,2048] x tile feeds K(x4) + Q(x2) + V(x2 first
      half) 512-wide matmuls -- 8 PSUM accumulator banks, x read once.
  1b: V second half from re-read [128,1024] x tiles (2 banks).
  2:  per q-tile: logits -> one fused DVE add (PSUM f32 + mask -> bf16, which
      applies the reference's bf16 logits rounding) -> row max -> exp (ACT,
      bias=-max, accumulates row sum) -> W^T via DMA xbar transpose -> PV,
      software-pipelined so PE stays busy across q-tiles.
"""

import numpy as np
import ml_dtypes

import concourse.bass as bass
import concourse.tile as tile
from concourse import bacc, mybir
from concourse.bass_utils import run_bass_kernel_spmd

B, S, D, DH = 4, 2048, 4096, 128
SQ = S // 2          # q rows per core
N_CORES = 8
D_CH = D // 128      # 32 contraction chunks
QT_TILES = SQ // 128 # 8 q row tiles
K_CH = S // 128      # 16 key chunks for PV

BF16 = mybir.dt.bfloat16
F32 = mybir.dt.float32


def build_nc():
    nc = bacc.Bacc(None)

    xT = nc.dram_tensor("xT", [D, S], BF16, kind="ExternalInput")
    mask = nc.dram_tensor("mask", [SQ, S], BF16, kind="ExternalInput")
    # weights pre-tiled on host: w[p, i, m] = W[m, i*128+p]
    wqT = nc.dram_tensor("wqT", [128, D_CH, DH], BF16, kind="ExternalInput")
    wkT = nc.dram_tensor("wkT", [128, D_CH, DH], BF16, kind="ExternalInput")
    wvT = nc.dram_tensor("wvT", [128, D_CH, DH], BF16, kind="ExternalInput")
    bq = nc.dram_tensor("bq", [DH, 1], F32, kind="ExternalInput")
    bk = nc.dram_tensor("bk", [DH, 1], F32, kind="ExternalInput")
    bv = nc.dram_tensor("bv", [DH, 1], F32, kind="ExternalInput")
    out = nc.dram_tensor("out", [SQ, DH], BF16, kind="ExternalOutput")

    with tile.TileContext(nc) as tc:
        with (
            tc.tile_pool(name="weights", bufs=1) as wpool,
            tc.tile_pool(name="persist", bufs=1) as persist,
        ):
            w_sb = {}
            for name, ext in (("q", wqT), ("k", wkT), ("v", wvT)):
                w_sb[name] = wpool.tile([128, D_CH, DH], BF16, tag=f"w{name}",
                                        name=f"w{name}")
            # weights + biases on the scalar HWDGE queue so the sync queue
            # streams x tiles from instruction 0
            for sl in range(4):
                for name, ext in (("k", wkT), ("q", wqT), ("v", wvT)):
                    ss = np.s_[:, sl * 8:(sl + 1) * 8, :]
                    nc.scalar.dma_start(out=w_sb[name][ss], in_=ext[ss])
            b_sb = {}
            for name, ext in (("q", bq), ("k", bk), ("v", bv)):
                t = wpool.tile([DH, 1], F32, tag=f"b{name}")
                nc.scalar.dma_start(out=t[:], in_=ext[:])
                b_sb[name] = t

            kt_sb = persist.tile([DH, S], BF16, tag="kt")    # K^T
            vt_sb = persist.tile([DH, S], BF16, tag="vt")    # V^T (pre-transpose)
            qt_sb = persist.tile([DH, SQ], BF16, tag="qt")   # Q^T
            v_sb = persist.tile([128, K_CH, DH], BF16, tag="v")  # V[kc*128+p, d]

            # --- phase 1a: K (full) + Q (full) + V (first half), 8 PSUM banks ---
            with tc.tile_pool(name="ppsum_a", bufs=1, space="PSUM") as ppa:
                acc = {}
                for tag, n in (("pk", 4), ("pq", 2), ("pv", 2)):
                    for j in range(n):
                        acc[(tag, j)] = ppa.tile([DH, 512], F32, tag=f"{tag}{j}", name=f"{tag}{j}")
                with tc.tile_pool(name="xin_a", bufs=6) as xpa:
                    for i in range(D_CH):
                        xt = xpa.tile([128, S], BF16, tag="xt")
                        nc.sync.dma_start(out=xt[:], in_=xT[i * 128:(i + 1) * 128, :])
                        st = dict(start=(i == 0), stop=(i == D_CH - 1))
                        for j in range(4):
                            nc.tensor.matmul(acc[("pk", j)][:], lhsT=w_sb["k"][:, i, :],
                                             rhs=xt[:, j * 512:(j + 1) * 512], **st)
                        for j in range(2):
                            nc.tensor.matmul(acc[("pq", j)][:], lhsT=w_sb["q"][:, i, :],
                                             rhs=xt[:, j * 512:(j + 1) * 512], **st)
                        for j in range(2):
                            nc.tensor.matmul(acc[("pv", j)][:], lhsT=w_sb["v"][:, i, :],
                                             rhs=xt[:, j * 512:(j + 1) * 512], **st)
                for j in range(4):
                    sl = np.s_[:, j * 512:(j + 1) * 512]
                    nc.vector.tensor_scalar_add(kt_sb[sl], acc[("pk", j)][:], b_sb["k"][:])
                for j in range(2):
                    sl = np.s_[:, j * 512:(j + 1) * 512]
                    nc.vector.tensor_scalar_add(qt_sb[sl], acc[("pq", j)][:], b_sb["q"][:])
                    nc.vector.tensor_scalar_add(vt_sb[sl], acc[("pv", j)][:], b_sb["v"][:])

            # --- phase 2: attention + V second half, software-pipelined ---
            with (
                tc.tile_pool(name="attn_sb", bufs=3) as apool,
                tc.tile_pool(name="wt_sb", bufs=6) as wtpool,
                tc.tile_pool(name="mask_sb", bufs=4) as mpool,
                tc.tile_pool(name="stats", bufs=8) as stat,
                tc.tile_pool(name="l_psum", bufs=1, space="PSUM") as lpool,
                tc.tile_pool(name="o_psum", bufs=2, space="PSUM") as opool,
                tc.tile_pool(name="ppsum_b", bufs=1, space="PSUM") as ppb,
                tc.tile_pool(name="xin_b", bufs=4) as xpb,
                tc.tile_pool(name="out_sb", bufs=2) as ospool,
            ):
                accv = [ppb.tile([DH, 512], F32, tag=f"pv2{j}", name=f"pv2{j}")
                        for j in range(2)]

                def v2_chunk(ci):
                    # d-chunks of the V second-half projection; x tiles stream
                    # on the sync queue inside the attention window
                    for i in range(ci * 8, ci * 8 + 8):
                        xt = xpb.tile([128, SQ], BF16, tag="xt2")
                        nc.sync.dma_start(out=xt[:], in_=xT[i * 128:(i + 1) * 128, SQ:])
                        st = dict(start=(i == 0), stop=(i == D_CH - 1))
                        for j in range(2):
                            nc.tensor.matmul(accv[j][:], lhsT=w_sb["v"][:, i, :],
                                             rhs=xt[:, j * 512:(j + 1) * 512], **st)
                    if ci == 3:
                        for j in range(2):
                            sl = np.s_[:, SQ + j * 512:SQ + (j + 1) * 512]
                            nc.vector.tensor_scalar_add(vt_sb[sl], accv[j][:], b_sb["v"][:])
                        # V^T -> V via DMA xbar transpose (SBUF->SBUF)
                        nc.sync.dma_start_transpose(out=v_sb[:], in_=vt_sb[:])

                pv_args = {}

                # Causal skip with an SPMD-uniform graph: q-tile qt attends to
                # own-half key chunks 0..qt (chunks >qt are strictly above the
                # diagonal -> fully masked -> exp 0 -> contribute 0) plus all 8
                # other-half chunks. For h=0 cores the other half is entirely
                # masked (the per-core mask data zeroes it); for h=1 it is the
                # entirely-visible past. Only the diagonal chunk and the other
                # half ever need mask values; own chunks <qt are mask-free.
                def softmax_stage(qt):
                    qsl = np.s_[:, qt * 128:(qt + 1) * 128]
                    own = (qt + 1) * 128        # own-half extent in keys
                    ext = own + SQ              # total computed key extent
                    nch = qt + 1 + 8            # chunks of 128 computed
                    qrows = np.s_[qt * 128:(qt + 1) * 128]
                    mask_d = mpool.tile([128, SQ], BF16, tag="mask_d")
                    nc.gpsimd.dma_start(out=mask_d[:, :own], in_=mask[qrows, :own])
                    mask_o = mpool.tile([128, SQ], BF16, tag="mask_o")
                    nc.gpsimd.dma_start(out=mask_o[:], in_=mask[qrows, SQ:])

                    pl = lpool.tile([128, SQ], F32, tag="pl")
                    for lo in range(0, own, 512):
                        w = min(512, own - lo)
                        nc.tensor.matmul(pl[:, lo:lo + w], lhsT=qt_sb[qsl],
                                         rhs=kt_sb[:, lo:lo + w], start=True, stop=True)
                    plo = lpool.tile([128, SQ], F32, tag="plo")
                    for n in range(2):
                        nc.tensor.matmul(plo[:, n * 512:(n + 1) * 512], lhsT=qt_sb[qsl],
                                         rhs=kt_sb[:, SQ + n * 512:SQ + (n + 1) * 512],
                                         start=True, stop=True)

                    # compact lm: [0,own) = own-half, [own, ext) = other half
                    # one fused DVE add per half: f32 PSUM + bf16 mask -> bf16
                    # (the bf16 rounding matches the reference's logit dtype)
                    lm = apool.tile([128, S], BF16, tag="lm")
                    nc.vector.tensor_add(lm[:, :own], pl[:, :own], mask_d[:, :own])
                    nc.vector.tensor_add(lm[:, own:ext], plo[:], mask_o[:])
                    negmax = stat.tile([128, 1], F32, tag="negmax")
                    nc.vector.reduce_max(out=negmax[:], in_=lm[:, :ext],
                                         axis=mybir.AxisListType.X, negate=True)
                    w_t = apool.tile([128, S], BF16, tag="w")
                    sumexp = stat.tile([128, 1], F32, tag="sumexp")
                    nc.scalar.activation(
                        out=w_t[:, :ext], in_=lm[:, :ext],
                        func=mybir.ActivationFunctionType.Exp,
                        bias=negmax[:], scale=1.0, accum_out=sumexp[:],
                    )
                    wt_t = wtpool.tile([128, K_CH, 128], BF16, tag="wt")
                    nc.sync.dma_start_transpose(out=wt_t[:, :nch, :], in_=w_t[:, :ext])
                    pv_args[qt] = (wt_t, sumexp, nch)

                def pv_stage(qt):
                    wt_t, sumexp, nch = pv_args.pop(qt)
                    rsum = stat.tile([128, 1], F32, tag="rsum")
                    nc.vector.reciprocal(rsum[:], sumexp[:])
                    po = opool.tile([128, DH], F32, tag="po")
                    for c in range(nch):
                        vc = c if c <= qt else 8 + (c - qt - 1)
                        nc.tensor.matmul(po[:], lhsT=wt_t[:, c, :], rhs=v_sb[:, vc, :],
                                         start=(c == 0), stop=(c == nch - 1))
                    o_sb = ospool.tile([128, DH], BF16, tag="o")
                    nc.vector.tensor_scalar_mul(o_sb[:], po[:], rsum[:])
                    nc.gpsimd.dma_start(out=out[qt * 128:(qt + 1) * 128, :], in_=o_sb[:])

                DEPTH = 4
                for qt in range(QT_TILES):
                    softmax_stage(qt)
                    if qt < 4:
                        v2_chunk(qt)
                    if qt >= DEPTH:
                        pv_stage(qt - DEPTH)
                for qt in range(QT_TILES - DEPTH, QT_TILES):
                    pv_stage(qt)

    nc.finalize()
    return nc


def shard_inputs(x, attn_mask, Wq, bq, Wk, bk, Wv, bv):
    """Host-side shard prep. Returns in_maps for cores 0..7."""
    bf = ml_dtypes.bfloat16
    xb = np.asarray(x).astype(bf)                   # cast first, like the reference
    mask_f = np.asarray(attn_mask)

    def tile_w(W):
        # [DH, D] -> [128, D_CH, DH] with w[p, i, m] = W[m, i*128+p]
        WT = np.asarray(W).astype(bf).T.reshape(D_CH, 128, DH)
        return np.ascontiguousarray(WT.transpose(1, 0, 2))

    wqt, wkt, wvt = tile_w(Wq), tile_w(Wk), tile_w(Wv)
    bqc = np.asarray(bq).astype(bf).astype(np.float32).reshape(DH, 1)
    bkc = np.asarray(bk).astype(bf).astype(np.float32).reshape(DH, 1)
    bvc = np.asarray(bv).astype(bf).astype(np.float32).reshape(DH, 1)

    in_maps = []
    for c in range(N_CORES):
        b, h = divmod(c, 2)
        if h == 0:
            perm = np.arange(S)
        else:
            perm = np.concatenate([np.arange(SQ, S), np.arange(0, SQ)])
        xT = np.ascontiguousarray(xb[b][perm].T)                     # [D, S]
        msk = np.ascontiguousarray(
            mask_f[h * SQ:(h + 1) * SQ][:, perm].astype(bf))          # [SQ, S]
        in_maps.append({
            "xT": xT, "mask": msk,
            "wqT": wqt, "wkT": wkt, "wvT": wvt,
            "bq": bqc, "bk": bkc, "bv": bvc,
        })
    return in_maps


_NC_CACHE = {}


def kernel(x, attn_mask, Wq, bq, Wk, bk, Wv, bv):
    if "nc" not in _NC_CACHE:
        _NC_CACHE["nc"] = build_nc()
    nc = _NC_CACHE["nc"]
    in_maps = shard_inputs(x, attn_mask, Wq, bq, Wk, bk, Wv, bv)
    res = run_bass_kernel_spmd(nc, in_maps, list(range(N_CORES)))
    out = np.empty((B, S, DH), dtype=ml_dtypes.bfloat16)
    for c in range(N_CORES):
        b, h = divmod(c, 2)
        out[b, h * SQ:(h + 1) * SQ, :] = res.results[c]["out"]
    return out



# revision 2
# speedup vs baseline: 1.0132x; 1.0132x over previous
"""Trainium2 Bass kernel for a single attention head (B=4, S=2048, D=4096, DH=128).

Sharding: 8 cores = (batch b, half h). Core (b,h) owns the INTERLEAVED q-tile
set {t : t % 2 == h} of batch b (8 tiles of 128 rows). Host permutes x columns
(and mask/key columns) so the core's own tiles come first:
  perm = [h, 2+h, 4+h, ..., 14+h,  1-h, 3-h, ..., 15-(h? ... other parity)]
This balances causal attention work: slot j (local q-tile j = global tile
2j+h) attends own-positions 0..j and other-positions 8..8+j -- a uniform
graph of sum 2(j+1) = 72 key-chunks per core (vs 100 for a contiguous split).

Phases:
  1a: one pass over x [D, 2048] (16MB): K full (4 PSUM banks) + Q own half
      (2) + V first half / own positions (2). x second-half columns are also
      copied SBUF->SBUF into a resident buffer x2 for phase 1b.
  1b: V other positions from resident x2 (64 matmuls, no DMA dependency),
      split in two 512-col stages so PV can start early.
  2:  per slot j (smallest first): logits (own + other ranges) -> fused
      DVE add(mask)+rowmax per 512 group -> reduce to -max -> exp (ACT,
      accum rowsum) -> W^T via DMA xbar transpose -> PV -> scale -> out.
      1b stages and pv stages interleave with softmax stages so PE never
      waits on the softmax chain.
"""

import numpy as np
import ml_dtypes

import concourse.bass as bass
import concourse.tile as tile
from concourse import bacc, mybir
from concourse.bass_utils import run_bass_kernel_spmd

B, S, D, DH = 4, 2048, 4096, 128
SQ = S // 2          # own q rows per core
N_CORES = 8
D_CH = D // 128      # 32 contraction chunks
NT = 8               # local q tiles (slots)

BF16 = mybir.dt.bfloat16
F32 = mybir.dt.float32
NEG_BIG = -3.0e38


def build_nc():
    nc = bacc.Bacc(None)

    xT = nc.dram_tensor("xT", [D, S], BF16, kind="ExternalInput")
    mask = nc.dram_tensor("mask", [128, 256], BF16, kind="ExternalInput")
    # weights pre-tiled on host: w[p, i, m] = W[m, i*128+p]
    wqT = nc.dram_tensor("wqT", [128, D_CH, DH], BF16, kind="ExternalInput")
    wkT = nc.dram_tensor("wkT", [128, D_CH, DH], BF16, kind="ExternalInput")
    wvT = nc.dram_tensor("wvT", [128, D_CH, DH], BF16, kind="ExternalInput")
    bq = nc.dram_tensor("bq", [DH, 1], F32, kind="ExternalInput")
    bk = nc.dram_tensor("bk", [DH, 1], F32, kind="ExternalInput")
    bv = nc.dram_tensor("bv", [DH, 1], F32, kind="ExternalInput")
    out = nc.dram_tensor("out", [SQ, DH], BF16, kind="ExternalOutput")

    with tile.TileContext(nc) as tc:
        with (
            tc.tile_pool(name="weights", bufs=1) as wpool,
            tc.tile_pool(name="persist", bufs=1) as persist,
        ):
            w_sb = {}
            for name in ("q", "k", "v"):
                w_sb[name] = wpool.tile([128, D_CH, DH], BF16, tag=f"w{name}",
                                        name=f"w{name}")
            # weight slices ride the sync queue interleaved with the x
            # stream (see 1a loop) so they do not burst-starve it; the
            # scalar queue stays empty until phase 2
            W_EXT = {"q": wqT, "k": wkT, "v": wvT}

            def w_slice(g):
                ss = np.s_[:, g * 4:(g + 1) * 4, :]
                for name in ("k", "q", "v"):
                    nc.sync.dma_start(out=w_sb[name][ss], in_=W_EXT[name][ss])
            b_sb = {}
            for name, ext in (("q", bq), ("k", bk), ("v", bv)):
                t = wpool.tile([DH, 1], F32, tag=f"b{name}")
                nc.sync.dma_start(out=t[:], in_=ext[:])
                b_sb[name] = t
            w_slice(0)
            w_slice(1)
            mk = persist.tile([128, 256], BF16, tag="mk")
            nc.gpsimd.dma_start(out=mk[:], in_=mask[:])

            kt_sb = persist.tile([DH, S], BF16, tag="kt")    # K^T all keys
            vt_sb = persist.tile([DH, S], BF16, tag="vt")    # V^T staging
            qt_sb = persist.tile([DH, SQ], BF16, tag="qt")   # Q^T own
            v_sb = persist.tile([128, 2 * NT, DH], BF16, tag="v")  # V[pos*128+p, d]
            x2_sb = persist.tile([128, D_CH, SQ], BF16, tag="x2")  # x cols SQ:

            # --- phase 1a: K full + Q own + V own positions; 8 PSUM banks ---
            with tc.tile_pool(name="ppsum_a", bufs=1, space="PSUM") as ppa:
                acc = {}
                for tag, n in (("pk", 4), ("pq", 2), ("pv", 2)):
                    for j in range(n):
                        acc[(tag, j)] = ppa.tile([DH, 512], F32, tag=f"{tag}{j}",
                                                 name=f"{tag}{j}")
                with tc.tile_pool(name="xin_a", bufs=6) as xpa:
                    for i in range(D_CH):
                        if i % 4 == 0 and 2 + i // 4 < 8:
                            w_slice(2 + i // 4)
                        xt = xpa.tile([128, S], BF16, tag="xt")
                        nc.sync.dma_start(out=xt[:], in_=xT[i * 128:(i + 1) * 128, :])
                        st = dict(start=(i == 0), stop=(i == D_CH - 1))
                        for j in range(4):
                            nc.tensor.matmul(acc[("pk", j)][:], lhsT=w_sb["k"][:, i, :],
                                             rhs=xt[:, j * 512:(j + 1) * 512], **st)
                        for j in range(2):
                            nc.tensor.matmul(acc[("pq", j)][:], lhsT=w_sb["q"][:, i, :],
                                             rhs=xt[:, j * 512:(j + 1) * 512], **st)
                        for j in range(2):
                            nc.tensor.matmul(acc[("pv", j)][:], lhsT=w_sb["v"][:, i, :],
                                             rhs=xt[:, j * 512:(j + 1) * 512], **st)
                        # second-half columns -> resident x2 for phase-1b V;
                        # gpsimd SWDGE paces these smoothly through 1a
                        nc.gpsimd.dma_start(out=x2_sb[:, i, :], in_=xt[:, SQ:])
                # drain: kt first (logits gate on it), then qt, then vt own
                for j in range(4):
                    sl = np.s_[:, j * 512:(j + 1) * 512]
                    nc.vector.tensor_scalar_add(kt_sb[sl], acc[("pk", j)][:], b_sb["k"][:])
                for j in range(2):
                    sl = np.s_[:, j * 512:(j + 1) * 512]
                    nc.vector.tensor_scalar_add(qt_sb[sl], acc[("pq", j)][:], b_sb["q"][:])
                for j in range(2):
                    sl = np.s_[:, j * 512:(j + 1) * 512]
                    nc.vector.tensor_scalar_add(vt_sb[sl], acc[("pv", j)][:], b_sb["v"][:])
                # V own positions 0..7
                nc.sync.dma_start_transpose(out=v_sb[:, 0:NT, :], in_=vt_sb[:, 0:SQ])

            # --- phase 2 (+1b interleaved) ---
            with (
                tc.tile_pool(name="lg_psum", bufs=4, space="PSUM") as lg,
                tc.tile_pool(name="v2_psum", bufs=2, space="PSUM") as pv2,
                tc.tile_pool(name="o_psum", bufs=2, space="PSUM") as opool,
                tc.tile_pool(name="lm_sb", bufs=1) as lmpool,
                tc.tile_pool(name="wt_sb", bufs=1) as wtpool,
                tc.tile_pool(name="stats", bufs=12) as stat,
                tc.tile_pool(name="out_sb", bufs=2) as ospool,
            ):
                def v2_compute(si):
                    lo = SQ + si * 512
                    accv = pv2.tile([DH, 512], F32, tag="pv2")
                    for i in range(D_CH):
                        nc.tensor.matmul(accv[:], lhsT=w_sb["v"][:, i, :],
                                         rhs=x2_sb[:, i, si * 512:(si + 1) * 512],
                                         start=(i == 0), stop=(i == D_CH - 1))
                    nc.vector.tensor_scalar_add(vt_sb[:, lo:lo + 512],
                                                accv[:], b_sb["v"][:])
                    nc.sync.dma_start_transpose(
                        out=v_sb[:, NT + si * 4:NT + (si + 1) * 4, :],
                        in_=vt_sb[:, lo:lo + 512])

                pv_args = {}

                def softmax_stage(j):
                    e = j + 1            # chunks per range
                    w = e * 128          # cols per range
                    qsl = np.s_[:, j * 128:(j + 1) * 128]

                    # Only the LAST chunk of each range carries mask values:
                    # own range ends in the causal-diagonal triangle chunk
                    # (mk[:, 0:128], same for every slot), the other range
                    # ends in the h-dependent all-0/all--1e9 chunk
                    # (mk[:, 128:256]). Every other chunk is pure past ->
                    # mask-free: just round psum f32 -> bf16, offloaded to
                    # the gpsimd vector engine to keep DVE off the chain.
                    lmt = lmpool.tile([128, 2 * w], BF16, tag=f"lm{j}")
                    for base, off, mcol in ((0, 0, 0), (SQ, w, 128)):
                        for g0 in range(0, w, 512):
                            gw = min(512, w - g0)
                            pg = lg.tile([128, 512], F32, tag="pg")
                            nc.tensor.matmul(pg[:, :gw], lhsT=qt_sb[qsl],
                                             rhs=kt_sb[:, base + g0:base + g0 + gw],
                                             start=True, stop=True)
                            last = g0 + gw == w
                            cp = gw - 128 if last else gw
                            if cp:
                                nc.vector.tensor_copy(
                                    lmt[:, off + g0:off + g0 + cp], pg[:, :cp])
                            if last:
                                # psum f32 + bf16 mask -> bf16 (the
                                # reference's bf16 logit rounding)
                                nc.vector.tensor_add(
                                    lmt[:, off + g0 + cp:off + g0 + gw],
                                    pg[:, cp:gw],
                                    mk[:, mcol:mcol + 128])
                    # single full-width rowmax of the ROUNDED bf16 logits
                    # (rounding can exceed the f32 max, so max follows it);
                    # one exp + one transpose per slot -- per-op fixed costs
                    # dominate at these sizes, fewer bigger ops win
                    negmax = stat.tile([128, 1], F32, tag="negmax")
                    nc.vector.reduce_max(out=negmax[:], in_=lmt[:, :2 * w],
                                         axis=mybir.AxisListType.X, negate=True)
                    w_t = lmpool.tile([128, 2 * w], BF16, tag=f"w{j}")
                    wt_t = wtpool.tile([128, 2 * e, 128], BF16, tag=f"wt{j}")
                    sumexp = stat.tile([128, 1], F32, tag="sumexp")
                    nc.scalar.activation(
                        out=w_t[:, :2 * w], in_=lmt[:, :2 * w],
                        func=mybir.ActivationFunctionType.Exp,
                        bias=negmax[:], scale=1.0, accum_out=sumexp[:])
                    nc.sync.dma_start_transpose(out=wt_t[:, :2 * e, :],
                                                in_=w_t[:, :2 * w])
                    pv_args[j] = (wt_t, sumexp, e)

                def pv_stage(j):
                    wt_t, sumexp, e = pv_args.pop(j)
                    rsum = stat.tile([128, 1], F32, tag="rsum")
                    nc.vector.reciprocal(rsum[:], sumexp[:])
                    po = opool.tile([128, DH], F32, tag="po")
                    for c in range(2 * e):
                        vc = c if c < e else NT + (c - e)
                        nc.tensor.matmul(po[:], lhsT=wt_t[:, c, :], rhs=v_sb[:, vc, :],
                                         start=(c == 0), stop=(c == 2 * e - 1))
                    o_sb = ospool.tile([128, DH], BF16, tag="o")
                    nc.vector.tensor_scalar_mul(o_sb[:], po[:], rsum[:])
                    nc.sync.dma_start(out=out[j * 128:(j + 1) * 128, :], in_=o_sb[:])

                # smallest slots first; v2 stages early (PV other-positions
                # ready by pv(0)); pv lags softmax by 3+ slots so each slot's
                # softmax chain hides under other slots' PE work
                softmax_stage(0)
                softmax_stage(1)
                v2_compute(0)
                softmax_stage(2)
                v2_compute(1)
                softmax_stage(3)
                pv_stage(0)
                softmax_stage(4)
                pv_stage(1)
                softmax_stage(5)
                pv_stage(2)
                softmax_stage(6)
                pv_stage(3)
                softmax_stage(7)
                pv_stage(4)
                pv_stage(5)
                pv_stage(6)
                pv_stage(7)

    nc.finalize()
    return nc


def shard_inputs(x, attn_mask, Wq, bq, Wk, bk, Wv, bv):
    """Host-side shard prep. Returns in_maps for cores 0..7."""
    bf = ml_dtypes.bfloat16
    xb = np.asarray(x).astype(bf)                   # cast first, like the reference
    mask_f = np.asarray(attn_mask)

    def tile_w(W):
        WT = np.asarray(W).astype(bf).T.reshape(D_CH, 128, DH)
        return np.ascontiguousarray(WT.transpose(1, 0, 2))

    wqt, wkt, wvt = tile_w(Wq), tile_w(Wk), tile_w(Wv)
    bqc = np.asarray(bq).astype(bf).astype(np.float32).reshape(DH, 1)
    bkc = np.asarray(bk).astype(bf).astype(np.float32).reshape(DH, 1)
    bvc = np.asarray(bv).astype(bf).astype(np.float32).reshape(DH, 1)

    in_maps = []
    for c in range(N_CORES):
        b, h = divmod(c, 2)
        own = np.concatenate([np.arange(t * 128, (t + 1) * 128)
                              for t in range(h, 16, 2)])
        oth = np.concatenate([np.arange(t * 128, (t + 1) * 128)
                              for t in range(1 - h, 16, 2)])
        perm = np.concatenate([own, oth])
        xTc = np.ascontiguousarray(xb[b][perm].T)                    # [D, S]
        tri = mask_f[:128, :128]                  # causal triangle (0/-1e9)
        hblk = np.full((128, 128), -1e9 if h == 0 else 0.0, dtype=np.float32)
        msk = np.ascontiguousarray(
            np.concatenate([tri, hblk], axis=1).astype(bf))  # [128, 256]
        in_maps.append({
            "xT": xTc, "mask": msk,
            "wqT": wqt, "wkT": wkt, "wvT": wvt,
            "bq": bqc, "bk": bkc, "bv": bvc,
        })
    return in_maps


_NC_CACHE = {}


def kernel(x, attn_mask, Wq, bq, Wk, bk, Wv, bv):
    if "nc" not in _NC_CACHE:
        _NC_CACHE["nc"] = build_nc()
    nc = _NC_CACHE["nc"]
    in_maps = shard_inputs(x, attn_mask, Wq, bq, Wk, bk, Wv, bv)
    res = run_bass_kernel_spmd(nc, in_maps, list(range(N_CORES)))
    out = np.empty((B, S, DH), dtype=ml_dtypes.bfloat16)
    for c in range(N_CORES):
        b, h = divmod(c, 2)
        for j in range(NT):
            t = 2 * j + h
            out[b, t * 128:(t + 1) * 128, :] = res.results[c]["out"][j * 128:(j + 1) * 128]
    return out


# revision 3
# speedup vs baseline: 1.2483x; 1.2320x over previous
"""Trainium2 Bass kernel for a single attention head (B=4, S=2048, D=4096, DH=128).

Sharding: 8 cores = (batch b, half h). Core (b,h) owns the INTERLEAVED q-tile
set {t : t % 2 == h} of batch b (8 tiles of 128 rows). Host permutes x columns
(and mask/key columns) so the core's own tiles come first:
  perm = [h, 2+h, 4+h, ..., 14+h,  1-h, 3-h, ..., 15-(h? ... other parity)]
This balances causal attention work: slot j (local q-tile j = global tile
2j+h) attends own-positions 0..j and other-positions 8..8+j -- a uniform
graph of sum 2(j+1) = 72 key-chunks per core (vs 100 for a contiguous split).

Phases:
  1a: one pass over x [D, 2048] (16MB): K full (4 PSUM banks) + Q own half
      (2) + V first half / own positions (2). x second-half columns are also
      copied SBUF->SBUF into a resident buffer x2 for phase 1b.
  1b: V other positions from resident x2 (64 matmuls, no DMA dependency),
      split in two 512-col stages so PV can start early.
  2:  per slot j (smallest first): logits (own + other ranges) -> fused
      DVE add(mask)+rowmax per 512 group -> reduce to -max -> exp (ACT,
      accum rowsum) -> W^T via DMA xbar transpose -> PV -> scale -> out.
      1b stages and pv stages interleave with softmax stages so PE never
      waits on the softmax chain.
"""

import numpy as np
import ml_dtypes

import concourse.bass as bass
import concourse.tile as tile
from concourse import bacc, mybir
from concourse.bass_utils import run_bass_kernel_spmd

B, S, D, DH = 4, 2048, 4096, 128
SQ = S // 2          # own q rows per core
N_CORES = 8
D_CH = D // 128      # 32 contraction chunks
NT = 8               # local q tiles (slots)

BF16 = mybir.dt.bfloat16
F32 = mybir.dt.float32
NEG_BIG = -3.0e38


def build_nc():
    nc = bacc.Bacc(None)

    xT = nc.dram_tensor("xT", [D, S], BF16, kind="ExternalInput")
    mask = nc.dram_tensor("mask", [128, 256], BF16, kind="ExternalInput")
    # weights pre-tiled on host: w[p, i, m] = W[m, i*128+p]
    wqT = nc.dram_tensor("wqT", [128, D_CH, DH], BF16, kind="ExternalInput")
    wkT = nc.dram_tensor("wkT", [128, D_CH, DH], BF16, kind="ExternalInput")
    wvT = nc.dram_tensor("wvT", [128, D_CH, DH], BF16, kind="ExternalInput")
    bq = nc.dram_tensor("bq", [DH, 1], F32, kind="ExternalInput")
    bk = nc.dram_tensor("bk", [DH, 1], F32, kind="ExternalInput")
    bv = nc.dram_tensor("bv", [DH, 1], F32, kind="ExternalInput")
    out = nc.dram_tensor("out", [SQ, DH], BF16, kind="ExternalOutput")

    with tile.TileContext(nc) as tc:
        with (
            tc.tile_pool(name="weights", bufs=1) as wpool,
            tc.tile_pool(name="persist", bufs=1) as persist,
        ):
            w_sb = {}
            for name in ("q", "k", "v"):
                w_sb[name] = wpool.tile([128, D_CH, DH], BF16, tag=f"w{name}",
                                        name=f"w{name}")
            # weight slices ride the sync queue interleaved with the x
            # stream (see 1a loop) so they do not burst-starve it; the
            # scalar queue stays empty until phase 2
            W_EXT = {"q": wqT, "k": wkT, "v": wvT}

            def w_slice(g):
                ss = np.s_[:, g * 4:(g + 1) * 4, :]
                for name in ("k", "q", "v"):
                    nc.sync.dma_start(out=w_sb[name][ss], in_=W_EXT[name][ss])
            b_sb = {}
            for name, ext in (("q", bq), ("k", bk), ("v", bv)):
                t = wpool.tile([DH, 1], F32, tag=f"b{name}")
                nc.sync.dma_start(out=t[:], in_=ext[:])
                b_sb[name] = t
            w_slice(0)
            w_slice(1)
            mk = persist.tile([128, 256], BF16, tag="mk")
            nc.gpsimd.dma_start(out=mk[:], in_=mask[:])

            kt_sb = persist.tile([DH, S], BF16, tag="kt")    # K^T all keys
            vt_sb = persist.tile([DH, S], BF16, tag="vt")    # V^T staging
            qt_sb = persist.tile([DH, SQ], BF16, tag="qt")   # Q^T own
            v_sb = persist.tile([128, 2 * NT, DH], BF16, tag="v")  # V[pos*128+p, d]
            x2_sb = persist.tile([128, D_CH, SQ], BF16, tag="x2")  # x cols SQ:

            # --- phase 1a: K full + Q own + V own positions; 8 PSUM banks ---
            with tc.tile_pool(name="ppsum_a", bufs=1, space="PSUM") as ppa:
                acc = {}
                for tag, n in (("pk", 4), ("pq", 2), ("pv", 2)):
                    for j in range(n):
                        acc[(tag, j)] = ppa.tile([DH, 512], F32, tag=f"{tag}{j}",
                                                 name=f"{tag}{j}")
                with tc.tile_pool(name="xin_a", bufs=6) as xpa:
                    for i in range(D_CH):
                        if i % 4 == 0 and 2 + i // 4 < 8:
                            w_slice(2 + i // 4)
                        xt = xpa.tile([128, S], BF16, tag="xt")
                        nc.sync.dma_start(out=xt[:], in_=xT[i * 128:(i + 1) * 128, :])
                        st = dict(start=(i == 0), stop=(i == D_CH - 1))
                        for j in range(4):
                            nc.tensor.matmul(acc[("pk", j)][:], lhsT=w_sb["k"][:, i, :],
                                             rhs=xt[:, j * 512:(j + 1) * 512], **st)
                        for j in range(2):
                            nc.tensor.matmul(acc[("pq", j)][:], lhsT=w_sb["q"][:, i, :],
                                             rhs=xt[:, j * 512:(j + 1) * 512], **st)
                        for j in range(2):
                            nc.tensor.matmul(acc[("pv", j)][:], lhsT=w_sb["v"][:, i, :],
                                             rhs=xt[:, j * 512:(j + 1) * 512], **st)
                        # second-half columns -> resident x2 for phase-1b V;
                        # gpsimd SWDGE paces these smoothly through 1a
                        nc.gpsimd.dma_start(out=x2_sb[:, i, :], in_=xt[:, SQ:])
                # drain: kt first (logits gate on it), then qt, then vt own
                for j in range(4):
                    sl = np.s_[:, j * 512:(j + 1) * 512]
                    nc.vector.tensor_scalar_add(kt_sb[sl], acc[("pk", j)][:], b_sb["k"][:])
                for j in range(2):
                    sl = np.s_[:, j * 512:(j + 1) * 512]
                    nc.vector.tensor_scalar_add(qt_sb[sl], acc[("pq", j)][:], b_sb["q"][:])
                for j in range(2):
                    sl = np.s_[:, j * 512:(j + 1) * 512]
                    nc.vector.tensor_scalar_add(vt_sb[sl], acc[("pv", j)][:], b_sb["v"][:])
                # V own positions 0..7
                nc.sync.dma_start_transpose(out=v_sb[:, 0:NT, :], in_=vt_sb[:, 0:SQ])

            # --- phase 2 (+1b interleaved) ---
            with (
                tc.tile_pool(name="lg_psum", bufs=4, space="PSUM") as lg,
                tc.tile_pool(name="v2_psum", bufs=2, space="PSUM") as pv2,
                tc.tile_pool(name="o_psum", bufs=2, space="PSUM") as opool,
                tc.tile_pool(name="lm_sb", bufs=1) as lmpool,
                tc.tile_pool(name="wt_sb", bufs=1) as wtpool,
                tc.tile_pool(name="stats", bufs=12) as stat,
                tc.tile_pool(name="out_sb", bufs=2) as ospool,
            ):
                def v2_compute(si):
                    lo = SQ + si * 512
                    accv = pv2.tile([DH, 512], F32, tag="pv2")
                    for i in range(D_CH):
                        nc.tensor.matmul(accv[:], lhsT=w_sb["v"][:, i, :],
                                         rhs=x2_sb[:, i, si * 512:(si + 1) * 512],
                                         start=(i == 0), stop=(i == D_CH - 1))
                    nc.vector.tensor_scalar_add(vt_sb[:, lo:lo + 512],
                                                accv[:], b_sb["v"][:])
                    nc.sync.dma_start_transpose(
                        out=v_sb[:, NT + si * 4:NT + (si + 1) * 4, :],
                        in_=vt_sb[:, lo:lo + 512])

                pv_args = {}

                def softmax_stage(j):
                    e = j + 1            # chunks per range
                    w = e * 128          # cols per range
                    qsl = np.s_[:, j * 128:(j + 1) * 128]

                    # Only the LAST chunk of each range carries mask values:
                    # own range ends in the causal-diagonal triangle chunk
                    # (mk[:, 0:128], same for every slot), the other range
                    # ends in the h-dependent all-0/all--1e9 chunk
                    # (mk[:, 128:256]). Every other chunk is pure past ->
                    # mask-free: just round psum f32 -> bf16, offloaded to
                    # the gpsimd vector engine to keep DVE off the chain.
                    lmt = lmpool.tile([128, 2 * w], BF16, tag=f"lm{j}")
                    for base, off, mcol in ((0, 0, 0), (SQ, w, 128)):
                        for g0 in range(0, w, 512):
                            gw = min(512, w - g0)
                            pg = lg.tile([128, 512], F32, tag="pg")
                            nc.tensor.matmul(pg[:, :gw], lhsT=qt_sb[qsl],
                                             rhs=kt_sb[:, base + g0:base + g0 + gw],
                                             start=True, stop=True)
                            last = g0 + gw == w
                            cp = gw - 128 if last else gw
                            if cp:
                                nc.vector.tensor_copy(
                                    lmt[:, off + g0:off + g0 + cp], pg[:, :cp])
                            if last:
                                # psum f32 + bf16 mask -> bf16 (the
                                # reference's bf16 logit rounding)
                                nc.vector.tensor_add(
                                    lmt[:, off + g0 + cp:off + g0 + gw],
                                    pg[:, cp:gw],
                                    mk[:, mcol:mcol + 128])
                    # single full-width rowmax of the ROUNDED bf16 logits
                    # (rounding can exceed the f32 max, so max follows it);
                    # one exp + one transpose per slot -- per-op fixed costs
                    # dominate at these sizes, fewer bigger ops win
                    negmax = stat.tile([128, 1], F32, tag="negmax")
                    nc.vector.reduce_max(out=negmax[:], in_=lmt[:, :2 * w],
                                         axis=mybir.AxisListType.X, negate=True)
                    w_t = lmpool.tile([128, 2 * w], BF16, tag=f"w{j}")
                    wt_t = wtpool.tile([128, 2 * e, 128], BF16, tag=f"wt{j}")
                    sumexp = stat.tile([128, 1], F32, tag="sumexp")
                    nc.scalar.activation(
                        out=w_t[:, :2 * w], in_=lmt[:, :2 * w],
                        func=mybir.ActivationFunctionType.Exp,
                        bias=negmax[:], scale=1.0, accum_out=sumexp[:])
                    nc.sync.dma_start_transpose(out=wt_t[:, :2 * e, :],
                                                in_=w_t[:, :2 * w])
                    pv_args[j] = (wt_t, sumexp, e)

                def pv_stage(j):
                    wt_t, sumexp, e = pv_args.pop(j)
                    rsum = stat.tile([128, 1], F32, tag="rsum")
                    nc.vector.reciprocal(rsum[:], sumexp[:])
                    po = opool.tile([128, DH], F32, tag="po")
                    for c in range(2 * e):
                        vc = c if c < e else NT + (c - e)
                        nc.tensor.matmul(po[:], lhsT=wt_t[:, c, :], rhs=v_sb[:, vc, :],
                                         start=(c == 0), stop=(c == 2 * e - 1))
                    o_sb = ospool.tile([128, DH], BF16, tag="o")
                    nc.vector.tensor_scalar_mul(o_sb[:], po[:], rsum[:])
                    nc.sync.dma_start(out=out[j * 128:(j + 1) * 128, :], in_=o_sb[:])

                # smallest slots first; v2 stages early (PV other-positions
                # ready by pv(0)); pv lags softmax by 3+ slots so each slot's
                # softmax chain hides under other slots' PE work
                softmax_stage(7)
                softmax_stage(6)
                v2_compute(0)
                softmax_stage(5)
                v2_compute(1)
                softmax_stage(4)
                pv_stage(7)
                softmax_stage(3)
                pv_stage(6)
                softmax_stage(2)
                pv_stage(5)
                softmax_stage(1)
                pv_stage(4)
                softmax_stage(0)
                pv_stage(3)
                pv_stage(2)
                pv_stage(1)
                pv_stage(0)

    nc.finalize()
    return nc


def shard_inputs(x, attn_mask, Wq, bq, Wk, bk, Wv, bv):
    """Host-side shard prep. Returns in_maps for cores 0..7."""
    bf = ml_dtypes.bfloat16
    xb = np.asarray(x).astype(bf)                   # cast first, like the reference
    mask_f = np.asarray(attn_mask)

    def tile_w(W):
        WT = np.asarray(W).astype(bf).T.reshape(D_CH, 128, DH)
        return np.ascontiguousarray(WT.transpose(1, 0, 2))

    wqt, wkt, wvt = tile_w(Wq), tile_w(Wk), tile_w(Wv)
    bqc = np.asarray(bq).astype(bf).astype(np.float32).reshape(DH, 1)
    bkc = np.asarray(bk).astype(bf).astype(np.float32).reshape(DH, 1)
    bvc = np.asarray(bv).astype(bf).astype(np.float32).reshape(DH, 1)

    in_maps = []
    for c in range(N_CORES):
        b, h = divmod(c, 2)
        own = np.concatenate([np.arange(t * 128, (t + 1) * 128)
                              for t in range(h, 16, 2)])
        oth = np.concatenate([np.arange(t * 128, (t + 1) * 128)
                              for t in range(1 - h, 16, 2)])
        perm = np.concatenate([own, oth])
        xTc = np.ascontiguousarray(xb[b][perm].T)                    # [D, S]
        tri = mask_f[:128, :128]                  # causal triangle (0/-1e9)
        hblk = np.full((128, 128), -1e9 if h == 0 else 0.0, dtype=np.float32)
        msk = np.ascontiguousarray(
            np.concatenate([tri, hblk], axis=1).astype(bf))  # [128, 256]
        in_maps.append({
            "xT": xTc, "mask": msk,
            "wqT": wqt, "wkT": wkt, "wvT": wvt,
            "bq": bqc, "bk": bkc, "bv": bvc,
        })
    return in_maps


_NC_CACHE = {}


def kernel(x, attn_mask, Wq, bq, Wk, bk, Wv, bv):
    if "nc" not in _NC_CACHE:
        _NC_CACHE["nc"] = build_nc()
    nc = _NC_CACHE["nc"]
    in_maps = shard_inputs(x, attn_mask, Wq, bq, Wk, bk, Wv, bv)
    res = run_bass_kernel_spmd(nc, in_maps, list(range(N_CORES)))
    out = np.empty((B, S, DH), dtype=ml_dtypes.bfloat16)
    for c in range(N_CORES):
        b, h = divmod(c, 2)
        for j in range(NT):
            t = 2 * j + h
            out[b, t * 128:(t + 1) * 128, :] = res.results[c]["out"][j * 128:(j + 1) * 128]
    return out
